# revision 10
# baseline (speedup 1.0000x reference)
"""Deformable PS-ROI pooling on Trainium2 (Bass/Tile), SPMD over 8 cores.

Strategy: data-parallel over ROIs (64 rois/core), feature map replicated in
DRAM in channel-last bf16 layout so each bilinear corner is one contiguous
512 B gather.  ROIs are processed in pairs: one 1568-descriptor dma_gather
per pair amortizes the fixed SWDGE descriptor-gen overhead.  Per (roi, bin)
the 4 samples x 4 corners = 16 gathered pixel vectors land on 16 SBUF
partitions; a block-diagonal bf16 mask matmul on the PE reduces them into
the [49, 256] output, with all bilinear / validity / 1-over-count factors
pre-folded into a per-partition scalar weight.  The second roi of a pair
lands 16 partitions later in the gather destination (784 = 6*128 + 16), so
it uses a shifted mask constant (cmB) and shifted weight transposes.
"""

import numpy as np
import ml_dtypes

import concourse.bass as bass
import concourse.bacc as bacc
import concourse.mybir as mybir
from concourse import tile
from concourse.bass_utils import run_bass_kernel_spmd

F32 = mybir.dt.float32
BF16 = mybir.dt.bfloat16
I32 = mybir.dt.int32
I16 = mybir.dt.int16
OP = mybir.AluOpType

N_CORES = 8
R = 64                  # rois per core
P = 7                   # pooled output size
NB = P * P              # 49 bins
CH = 256                # channels
H = W = 128             # feature map spatial
B = 2                   # batch
NPX = B * H * W         # 32768 flat pixels
TPB = 16                # terms (sample x corner) per bin
T = NB * TPB            # 784 terms per roi
NPAIR = R // 2          # rois are gathered in pairs (1568 descs/gather)
NCOL = 13               # gather dest cols per pair (1568 = 12*128 + 32)
NCH = 7                 # mask chunks per roi (each covers <=8 bins)
SCALE = 0.0625
TRANS_STD = 0.1
GP_BUFS = 8


def _floor(nc, pool, x, name):
    """floor(x) robust to convert rounding mode: returns (floor_f32, frac)."""
    xi = pool.tile([R, x.shape[1]], I32, tag=name + "_i")
    nc.vector.tensor_copy(xi[:, :], x)
    xf = pool.tile([R, x.shape[1]], F32, tag=name + "_f")
    nc.vector.tensor_copy(xf[:, :], xi[:, :])
    d = pool.tile([R, x.shape[1]], F32, tag=name + "_d")
    nc.vector.tensor_tensor(d[:, :], x, xf[:, :], OP.subtract)
    neg = pool.tile([R, x.shape[1]], F32, tag=name + "_n")
    nc.vector.tensor_scalar(neg[:, :], d[:, :], 0.0, None, OP.is_lt)
    fl = pool.tile([R, x.shape[1]], F32, tag=name + "_fl")
    nc.vector.tensor_tensor(fl[:, :], xf[:, :], neg[:, :], OP.subtract)
    fr = pool.tile([R, x.shape[1]], F32, tag=name + "_fr")
    nc.vector.tensor_tensor(fr[:, :], d[:, :], neg[:, :], OP.add)
    return fl[:, :], fr[:, :]


def build_program(reps: int = 1):
    nc = bacc.Bacc("TRN2", target_bir_lowering=False, debug=False, num_swdge_queues=4)
    nc.dynamic_dma_scratch_size = 2 ** 16

    data = nc.dram_tensor("data_t", [NPX, CH], BF16, kind="ExternalInput")
    rois_d = nc.dram_tensor("rois", [R, 5], F32, kind="ExternalInput")
    off_d = nc.dram_tensor("offs", [R, 2 * NB], F32, kind="ExternalInput")
    iopw_d = nc.dram_tensor("iota_pw", [R, NB], F32, kind="ExternalInput")
    ioph_d = nc.dram_tensor("iota_ph", [R, NB], F32, kind="ExternalInput")
    iden_d = nc.dram_tensor("identity", [R, R], F32, kind="ExternalInput")
    cmA_d = nc.dram_tensor("cmaskA", [128, NCH * NB], F32, kind="ExternalInput")
    cmB_d = nc.dram_tensor("cmaskB", [128, NCH * NB], F32, kind="ExternalInput")
    out_d = nc.dram_tensor("out", [R, NB * CH], F32, kind="ExternalOutput")

    with tile.TileContext(nc) as tc:
        with (
            tc.tile_pool(name="const", bufs=1) as cst,
            tc.tile_pool(name="work", bufs=1) as wk,
            tc.tile_pool(name="gp", bufs=GP_BUFS) as gp,
            tc.tile_pool(name="gwp", bufs=8) as gwp,
            tc.tile_pool(name="obp", bufs=6) as obp,
            tc.tile_pool(name="psp", bufs=3, space="PSUM") as psp,
            tc.tile_pool(name="pst", bufs=2, space="PSUM") as pst,
        ):
            # ---- load inputs / constants to SBUF ----
            rois = cst.tile([R, 5], F32)
            nc.sync.dma_start(rois[:, :], rois_d.ap())
            off = cst.tile([R, 2 * NB], F32)
            nc.sync.dma_start(off[:, :], off_d.ap())
            iopw = cst.tile([R, NB], F32)
            nc.sync.dma_start(iopw[:, :], iopw_d.ap())
            ioph = cst.tile([R, NB], F32)
            nc.sync.dma_start(ioph[:, :], ioph_d.ap())
            iden = cst.tile([R, R], F32)
            nc.sync.dma_start(iden[:, :], iden_d.ap())
            cmA = cst.tile([128, NCH * NB], F32)
            nc.sync.dma_start(cmA[:, :], cmA_d.ap())
            cmB = cst.tile([128, NCH * NB], F32)
            nc.sync.dma_start(cmB[:, :], cmB_d.ap())

            from contextlib import nullcontext
            loop_cm = tc.For_i(0, reps, 1) if reps > 1 else nullcontext()
            with loop_cm:
                # ---- phase A: per-roi coordinate math, roi on partition ----
                # round(rois[:,1:5]) = floor(x + 0.5)
                rr = wk.tile([R, 4], F32)
                nc.vector.tensor_scalar(rr[:, :], rois[:, 1:5], 0.5, None, OP.add)
                rnd, _ = _floor(nc, wk, rr[:, :], "rnd")

                # start/end in feature coords
                swsh = wk.tile([R, 2], F32)
                nc.vector.tensor_scalar(swsh[:, :], rnd[:, 0:2], SCALE, -0.5, OP.mult, OP.add)
                eweh = wk.tile([R, 2], F32)
                nc.vector.tensor_scalar(
                    eweh[:, :], rnd[:, 2:4], SCALE, SCALE - 0.5, OP.mult, OP.add
                )
                rwh0 = wk.tile([R, 2], F32)
                nc.vector.tensor_tensor(rwh0[:, :], eweh[:, :], swsh[:, :], OP.subtract)
                rwh = wk.tile([R, 2], F32)
                nc.vector.tensor_scalar(rwh[:, :], rwh0[:, :], 0.1, None, OP.max)
                bwh = wk.tile([R, 2], F32)
                nc.vector.tensor_scalar(bwh[:, :], rwh[:, :], 1.0 / P, None, OP.mult)
                swh = wk.tile([R, 2], F32)
                nc.vector.tensor_scalar(swh[:, :], bwh[:, :], 0.5, None, OP.mult)
                rwh01 = wk.tile([R, 2], F32)
                nc.vector.tensor_scalar(rwh01[:, :], rwh[:, :], TRANS_STD, None, OP.mult)
                ybase = wk.tile([R, 1], F32)
                nc.vector.tensor_scalar(ybase[:, :], rois[:, 0:1], float(H * W), None, OP.mult)

                # bin starts, shifted by learned offsets: [R, 49]
                def bin_start(iota, bcol, scol, tview, r01col, name):
                    t0 = wk.tile([R, NB], F32, tag=name + "0")
                    nc.vector.tensor_scalar(t0[:, :], iota, bcol, None, OP.mult)
                    t1 = wk.tile([R, NB], F32, tag=name + "1")
                    nc.vector.scalar_tensor_tensor(
                        t1[:, :], tview, r01col, t0[:, :], OP.mult, OP.add
                    )
                    t2 = wk.tile([R, NB], F32, tag=name + "2")
                    nc.vector.tensor_scalar(t2[:, :], t1[:, :], scol, None, OP.add)
                    return t2

                wstart = bin_start(
                    iopw[:, :], bwh[:, 0:1], swsh[:, 0:1], off[:, 0:NB],
                    rwh01[:, 0:1], "ws",
                )
                hstart = bin_start(
                    ioph[:, :], bwh[:, 1:2], swsh[:, 1:2], off[:, NB : 2 * NB],
                    rwh01[:, 1:2], "hs",
                )

                # sample positions [R, 98] = (bin, s)
                def samples(start, subcol, name):
                    s2 = wk.tile([R, 2 * NB], F32, tag=name)
                    v = s2[:, :].rearrange("p (b s) -> p b s", s=2)
                    su = start[:, :].rearrange("p b -> p b", ).unsqueeze(2)
                    nc.vector.tensor_copy(v[:, :, 0:1], su)
                    nc.vector.tensor_scalar(v[:, :, 1:2], su, subcol, None, OP.add)
                    return s2

                X2 = samples(wstart, swh[:, 0:1], "X2")
                Y2 = samples(hstart, swh[:, 1:2], "Y2")

                # per-axis: validity, clip, floor/frac, weight pairs, index pairs
                def axis_side(S2, lim, name):
                    # valid = (S2 >= -0.5) & (S2 <= lim + 0.5)
                    va = wk.tile([R, 2 * NB], F32, tag=name + "va")
                    nc.vector.tensor_scalar(va[:, :], S2[:, :], -0.5, None, OP.is_ge)
                    vv = wk.tile([R, 2 * NB], F32, tag=name + "vv")
                    nc.vector.scalar_tensor_tensor(
                        vv[:, :], S2[:, :], lim + 0.5, va[:, :], OP.is_le, OP.mult
                    )
                    cl = wk.tile([R, 2 * NB], F32, tag=name + "cl")
                    nc.vector.tensor_scalar(cl[:, :], S2[:, :], 0.0, lim, OP.max, OP.min)
                    flo, fra = _floor(nc, wk, cl[:, :], name + "fl")
                    # count over the 2 samples, per bin -> reciprocal (exact: 1 or .5)
                    cnt = wk.tile([R, NB], F32, tag=name + "ct")
                    vvv = vv[:, :].rearrange("p (b s) -> p b s", s=2)
                    nc.vector.tensor_tensor(
                        cnt[:, :].unsqueeze(2),
                        vvv[:, :, 0:1], vvv[:, :, 1:2], OP.add,
                    )
                    eq2 = wk.tile([R, NB], F32, tag=name + "e2")
                    nc.vector.tensor_scalar(eq2[:, :], cnt[:, :], 2.0, None, OP.is_equal)
                    rc = wk.tile([R, NB], F32, tag=name + "rc")
                    nc.vector.tensor_scalar(rc[:, :], eq2[:, :], -0.5, 1.0, OP.mult, OP.add)
                    # weight pair: w0 = v*(1-f)*rc, w1 = v*f*rc  [R, 196] = (bin, s, c)
                    rcb = rc[:, :].unsqueeze(2).broadcast_to([R, NB, 2])
                    vr = wk.tile([R, 2 * NB], F32, tag=name + "vr")
                    nc.vector.tensor_tensor(
                        vr[:, :].rearrange("p (b s) -> p b s", s=2), vvv, rcb, OP.mult
                    )
                    w1 = wk.tile([R, 2 * NB], F32, tag=name + "w1")
                    nc.vector.tensor_tensor(w1[:, :], vr[:, :], fra, OP.mult)
                    w0 = wk.tile([R, 2 * NB], F32, tag=name + "w0")
                    nc.vector.tensor_tensor(w0[:, :], vr[:, :], w1[:, :], OP.subtract)
                    W4 = wk.tile([R, 4 * NB], F32, tag=name + "W4")
                    W4v = W4[:, :].rearrange("p (b s c) -> p b s c", s=2, c=2)
                    w0v = w0[:, :].rearrange("p (b s) -> p b s", s=2).unsqueeze(3)
                    w1v = w1[:, :].rearrange("p (b s) -> p b s", s=2).unsqueeze(3)
                    nc.vector.tensor_copy(W4v[:, :, :, 0:1], w0v)
                    nc.vector.tensor_copy(W4v[:, :, :, 1:2], w1v)
                    # index pair: i0 = floor, i1 = min(floor+1, lim)
                    I4 = wk.tile([R, 4 * NB], F32, tag=name + "I4")
                    I4v = I4[:, :].rearrange("p (b s c) -> p b s c", s=2, c=2)
                    flv = flo.rearrange("p (b s) -> p b s", s=2).unsqueeze(3)
                    nc.vector.tensor_copy(I4v[:, :, :, 0:1], flv)
                    nc.vector.tensor_scalar(I4v[:, :, :, 1:2], flv, 1.0, lim, OP.add, OP.min)
                    return W4, I4

                WX4, XI4 = axis_side(X2, float(W - 1), "x")
                WY4, YI4 = axis_side(Y2, float(H - 1), "y")

                # y-side indices -> flat row base: b*H*W + y*W
                YIr = wk.tile([R, 4 * NB], F32)
                nc.vector.tensor_scalar(
                    YIr[:, :], YI4[:, :], float(W), ybase[:, :], OP.mult, OP.add
                )

                # weights expanded to full terms, bin-major [R, 784] = (b, h, y, s, x)
                Wt = wk.tile([R, T], F32)
                Wtv = Wt[:, :].rearrange(
                    "p (b h y s x) -> p b h y s x", h=2, y=2, s=2, x=2
                )
                WY4v = (
                    WY4[:, :].rearrange("p (b h y) -> p b h y", h=2, y=2)
                    .unsqueeze(4).unsqueeze(5)
                )
                for k in range(4):
                    s, x = k >> 1, k & 1
                    nc.vector.tensor_copy(Wtv[:, :, :, :, s : s + 1, x : x + 1], WY4v)
                WX4v = (
                    WX4[:, :].rearrange("p (b s x) -> p b s x", s=2, x=2)
                    .unsqueeze(2).unsqueeze(3)
                )
                for j in range(4):
                    h, y = j >> 1, j & 1
                    dstW = Wtv[:, :, h : h + 1, y : y + 1, :, :]
                    nc.vector.tensor_tensor(dstW, dstW, WX4v, OP.mult)

                # indices expanded lane-major [R, 784] = (h, y, s, x, b) so the
                # 16-lane-wrapped gather index tensor is a plain per-lane DMA.
                IDX2 = wk.tile([R, T], F32)
                IDX2v = IDX2[:, :].rearrange(
                    "p (h y s x b) -> p h y s x b", h=2, y=2, s=2, x=2
                )
                YIr2 = (
                    YIr[:, :].rearrange("p (b h y) -> p h y b", h=2, y=2)
                    .unsqueeze(3).unsqueeze(4)
                )
                for k in range(4):
                    s, x = k >> 1, k & 1
                    nc.vector.tensor_copy(IDX2v[:, :, :, s : s + 1, x : x + 1, :], YIr2)
                XI42 = (
                    XI4[:, :].rearrange("p (b s x) -> p s x b", s=2, x=2)
                    .unsqueeze(1).unsqueeze(2)
                )
                for j in range(4):
                    h, y = j >> 1, j & 1
                    dstI = IDX2v[:, h : h + 1, y : y + 1, :, :, :]
                    nc.vector.tensor_tensor(dstI, dstI, XI42, OP.add)

                # ---- phase B: int16 gather indices in dma_gather's 16-lane
                # layout (desc i of pair q reads IDXG[i%16, 98q + i//16]), and
                # weights transposed to the gather's landing partitions.
                IDX16 = wk.tile([R, T], I16)
                nc.vector.tensor_copy(IDX16[:, :], IDX2[:, :])
                IDXG = wk.tile([128, R * NB], I16)
                for t in range(16):
                    nc.sync.dma_start(
                        IDXG[t : t + 1, :], IDX16[:, t * NB : (t + 1) * NB]
                    )
                # Q7 tx/rx cpus each read their own 16-partition window of the
                # index tensor -> replicate lane group 0 across all 8 groups.
                for grp in range(1, 8):
                    nc.sync.dma_start(
                        IDXG[16 * grp : 16 * (grp + 1), :], IDXG[0:16, :]
                    )

                # WTg[p, c*32 + q] = weight of pair-q gather desc 128c + p.
                # Pair q = rois (2q, 2q+1); desc d<784 -> roi 2q term d,
                # d>=784 -> roi 2q+1 term d-784 (landing 16 partitions later).
                # PE transposes may only write at base partition 0, so the
                # B-roi pieces that land at partitions 16.. are staged at
                # partition 0 and shifted with one SBUF->SBUF DMA.
                WTg = wk.tile([128, NCOL * NPAIR], F32)
                WTgv = WTg[:, :].rearrange("p (c q) -> p c q", q=NPAIR)
                ST = wk.tile([112, NCH * NPAIR], F32)
                STv = ST[:, :].rearrange("p (c q) -> p c q", q=NPAIR)
                # staging col 6 rows 16.. feed WTg col-12 partitions 32..,
                # which are never gathered: keep them finite zeros.  (Memset
                # the whole group first; the WS copy then fills rows 0..15.)
                nc.vector.memset(STv[:, 6:7, :], 0.0)
                # (dst, dst col, partitions, term slice start, roi parity)
                specs = [("W", c, 128, 128 * c, 0) for c in range(6)]
                specs += [("W", 6 + k, 16, 128 * (k - 1) + 112, 1) for k in range(1, 6)]
                specs.append(("W", 12, 16, 752, 1))
                specs.append(("S", 0, 112, 0, 1))
                specs += [("S", k, 112, 128 * k, 1) for k in range(1, 6)]
                specs.append(("WS", 6, 16, 768, None))
                for (dst, c, n, t0, par) in specs:
                    ps = pst.tile([128, R], F32, tag="pstr")
                    nc.tensor.transpose(ps[0:n, :], Wt[:, t0 : t0 + n], iden[:, :])
                    if dst in ("W", "WS"):
                        nc.vector.tensor_copy(
                            WTgv[0:n, c : c + 1, :],
                            ps[0:n, (0 if dst == "WS" else par)::2].unsqueeze(1),
                        )
                    if dst in ("S", "WS"):
                        nc.vector.tensor_copy(
                            STv[0:n, c : c + 1, :], ps[0:n, 1::2].unsqueeze(1)
                        )
                nc.sync.dma_start(WTg[16:128, 6 * NPAIR :], ST[:, :])

                # ---- phase C: gather + weighted reduce, one pair at a time --
                out_v = out_d.ap().rearrange("r (b c) -> b r c", c=CH)
                for q in range(NPAIR):
                    gt = gp.tile([128, NCOL * CH], BF16)
                    if q < GP_BUFS:
                        # col 12 partitions 32.. stay stale; clear on each
                        # buffer's first use so masked-0 products are finite.
                        nc.vector.memset(gt[:, 12 * CH : 13 * CH], 0.0)
                    # Q7 idx scratch caps ~1024 descs per gather: split the
                    # pair's 1568 descs at the 1024 boundary (a multiple of
                    # 128, so the landing pattern is unchanged).
                    dest1 = gt[:, 0 : 8 * CH].rearrange("p (j f) -> p j f", f=CH)
                    nc.gpsimd.dma_gather(
                        dest1,
                        data.ap(),
                        IDXG[:, q * 2 * NB : q * 2 * NB + 64],
                        1024,
                        1024,
                        CH,
                        queue_num=(2 * q) % 4,
                    )
                    dest2 = gt[:, 8 * CH :].rearrange("p (j f) -> p j f", f=CH)
                    nc.gpsimd.dma_gather(
                        dest2,
                        data.ap(),
                        IDXG[:, q * 2 * NB + 64 : (q + 1) * 2 * NB],
                        544,
                        544,
                        CH,
                        queue_num=(2 * q + 1) % 4,
                    )
                    # weighted masks for both rois of the pair (bf16 for PE)
                    wmA = gwp.tile([128, NCH * NB], BF16, tag="wmA")
                    wtbA = WTgv[:, 0:NCH, q : q + 1].broadcast_to([128, NCH, NB])
                    nc.any.tensor_tensor(
                        wmA[:, :].rearrange("p (c j) -> p c j", j=NB),
                        cmA[:, :].rearrange("p (c j) -> p c j", j=NB),
                        wtbA,
                        OP.mult,
                    )
                    wmB = gwp.tile([128, NCH * NB], BF16, tag="wmB")
                    wtbB = WTgv[:, 6:NCOL, q : q + 1].broadcast_to([128, NCH, NB])
                    nc.any.tensor_tensor(
                        wmB[:, :].rearrange("p (c j) -> p c j", j=NB),
                        cmB[:, :].rearrange("p (c j) -> p c j", j=NB),
                        wtbB,
                        OP.mult,
                    )
                    psA = psp.tile([NB, CH], F32, tag="psA")
                    for c in range(NCH):
                        nc.tensor.matmul(
                            psA[:, :],
                            wmA[:, c * NB : (c + 1) * NB],
                            gt[:, c * CH : (c + 1) * CH],
                            start=(c == 0),
                            stop=(c == NCH - 1),
                        )
                    psB = psp.tile([NB, CH], F32, tag="psB")
                    for k in range(NCH):
                        nc.tensor.matmul(
                            psB[:, :],
                            wmB[:, k * NB : (k + 1) * NB],
                            gt[:, (6 + k) * CH : (7 + k) * CH],
                            start=(k == 0),
                            stop=(k == NCH - 1),
                        )
                    ob = obp.tile([NB, 2 * CH], F32)
                    nc.scalar.copy(ob[:, 0:CH], psA[:, :])
                    nc.scalar.copy(ob[:, CH : 2 * CH], psB[:, :])
                    nc.sync.dma_start(
                        out_v[:, 2 * q : 2 * q + 2, :],
                        ob[:, :].rearrange("p (r c) -> p r c", c=CH),
                    )

    nc.finalize()
    return nc


def host_constants():
    iopw = np.tile((np.arange(NB) % P).astype(np.float32), (R, 1))
    ioph = np.tile((np.arange(NB) // P).astype(np.float32), (R, 1))
    iden = np.eye(R, dtype=np.float32)
    # cmA: roi 2q at gather cols 0..6, bin slot = p//16 (8 bins/col).
    cmA = np.zeros((128, NCH * NB), dtype=np.float32)
    for j in range(6):
        for p in range(128):
            cmA[p, j * NB + 8 * j + p // 16] = 1.0
    for p in range(TPB):
        cmA[p, 6 * NB + 48] = 1.0
    # cmB: roi 2q+1 at gather cols 6..12, shifted one bin slot (16 parts).
    cmB = np.zeros((128, NCH * NB), dtype=np.float32)
    for k in range(NCH):
        for p in range(128):
            b = 8 * k + p // 16 - 1
            if 0 <= b < NB:
                cmB[p, k * NB + b] = 1.0
    return {
        "iota_pw": iopw, "iota_ph": ioph, "identity": iden,
        "cmaskA": cmA, "cmaskB": cmB,
    }


_cache = {}


def _program():
    if "nc" not in _cache:
        _cache["nc"] = build_program()
    return _cache["nc"]


def run(data, rois, offset, **spmd_kwargs):
    data = np.asarray(data, dtype=np.float32)
    rois = np.asarray(rois, dtype=np.float32)
    offset = np.asarray(offset, dtype=np.float32)
    n_rois = rois.shape[0]
    data_t = (
        np.ascontiguousarray(data.transpose(0, 2, 3, 1))
        .reshape(NPX, CH)
        .astype(ml_dtypes.bfloat16)
    )
    consts = host_constants()
    in_maps = []
    for c in range(N_CORES):
        sl = slice(c * R, (c + 1) * R)
        m = {
            "data_t": data_t,
            "rois": rois[sl],
            "offs": offset[sl].reshape(R, 2 * NB),
        }
        m.update(consts)
        in_maps.append(m)
    res = run_bass_kernel_spmd(
        _program(), in_maps, core_ids=list(range(N_CORES)), **spmd_kwargs
    )
    outs = np.concatenate([res.results[c]["out"] for c in range(N_CORES)], axis=0)
    out = outs.reshape(n_rois, NB, CH).transpose(0, 2, 1).reshape(n_rois, CH, P, P)
    return np.ascontiguousarray(out), res


def kernel(data, rois, offset):
    out, _ = run(data, rois, offset)
    return out


# revision 24
# speedup vs baseline: 1.4511x; 1.4511x over previous
"""Deformable PS-ROI pooling on Trainium2 (Bass/Tile), SPMD over 8 cores.

Strategy: data-parallel over ROIs (64 rois/core), feature map replicated in
DRAM in channel-last bf16 layout.  The two x-corners of a bilinear sample
are always adjacent pixels (x1, x1+1), so each gather descriptor fetches 2
contiguous pixels (1 KiB); HW gather cost is descriptor-bound, so this
halves gather time vs per-pixel descriptors.  ROIs are processed in pairs
(q, q+32): one 784-descriptor dma_gather per pair (fits the ~1024-desc Q7
idx scratch cap).  Descriptor i = 16*bin + 2*t3 + r (t3 = sample_h x
corner_y x sample_w, r = roi half) lands at partition 16*(bin%8)+2*t3+r,
so a single mask constant works for every column and the 16-lane index
tensor is a plain per-lane DMA.  A [128, 98] bf16 mask matmul on the PE
reduces each landing column into the pair's [98, 256] psum (rows 0..48
roi q, 49..97 roi q+32), with all bilinear / validity / 1-over-count
factors pre-folded into per-partition scalar weights (separate left- and
right-pixel variants, placed by stride-2-partition DMAs).
"""

import numpy as np
import ml_dtypes

import concourse.bass as bass
import concourse.bacc as bacc
import concourse.mybir as mybir
from concourse import tile
from concourse.bass_utils import run_bass_kernel_spmd

F32 = mybir.dt.float32
BF16 = mybir.dt.bfloat16
I32 = mybir.dt.int32
I16 = mybir.dt.int16
OP = mybir.AluOpType

N_CORES = 8
R = 64                  # rois per core
P = 7                   # pooled output size
NB = P * P              # 49 bins
CH = 256                # channels
H = W = 128             # feature map spatial
B = 2                   # batch
NPX = B * H * W         # 32768 flat pixels
PAD = 4                 # extra zero pixels (right-px overrun at x1=W-1)
T3 = 8                  # terms per (bin, roi): sample_h x corner_y x sample_w
TD = NB * T3            # 392 descriptor-terms per roi
ND = 2 * TD             # 784 descriptors per pair
NPAIR = R // 2
NCOL = 7                # gather dest cols per pair (784 = 6*128 + 16)
EL = 2 * CH             # elements per descriptor (2 pixels)
M2 = 2 * NB             # 98 psum rows per pair
SCALE = 0.0625
TRANS_STD = 0.1
GP_BUFS = 8


def _floor(nc, pool, x, name):
    """floor(x) robust to convert rounding mode: returns (floor_f32, frac)."""
    xi = pool.tile([R, x.shape[1]], I32, tag=name + "_i")
    nc.vector.tensor_copy(xi[:, :], x)
    xf = pool.tile([R, x.shape[1]], F32, tag=name + "_f")
    nc.vector.tensor_copy(xf[:, :], xi[:, :])
    d = pool.tile([R, x.shape[1]], F32, tag=name + "_d")
    nc.vector.tensor_tensor(d[:, :], x, xf[:, :], OP.subtract)
    neg = pool.tile([R, x.shape[1]], F32, tag=name + "_n")
    nc.vector.tensor_scalar(neg[:, :], d[:, :], 0.0, None, OP.is_lt)
    fl = pool.tile([R, x.shape[1]], F32, tag=name + "_fl")
    nc.vector.tensor_tensor(fl[:, :], xf[:, :], neg[:, :], OP.subtract)
    fr = pool.tile([R, x.shape[1]], F32, tag=name + "_fr")
    nc.vector.tensor_tensor(fr[:, :], d[:, :], neg[:, :], OP.add)
    return fl[:, :], fr[:, :]


def build_program(reps: int = 1, bench_mode: int = 0):
    """bench_mode: 0=full kernel, 1=gathers only (no reduce), 2=no gathers."""
    nc = bacc.Bacc("TRN2", target_bir_lowering=False, debug=False, num_swdge_queues=4)
    nc.dynamic_dma_scratch_size = 2 ** 16

    data = nc.dram_tensor("data_t", [NPX + PAD, CH], BF16, kind="ExternalInput")
    rois_d = nc.dram_tensor("rois", [R, 5], F32, kind="ExternalInput")
    off_d = nc.dram_tensor("offs", [R, 2 * NB], F32, kind="ExternalInput")
    iopw_d = nc.dram_tensor("iota_pw", [R, NB], F32, kind="ExternalInput")
    ioph_d = nc.dram_tensor("iota_ph", [R, NB], F32, kind="ExternalInput")
    iden_d = nc.dram_tensor("identity", [R, R], F32, kind="ExternalInput")
    cm_d = nc.dram_tensor("cmask", [128, NCOL * M2], F32, kind="ExternalInput")
    out_d = nc.dram_tensor("out", [R, NB * CH], F32, kind="ExternalOutput")

    with tile.TileContext(nc) as tc:
        with (
            tc.tile_pool(name="const", bufs=1) as cst,
            tc.tile_pool(name="work", bufs=1) as wk,
            tc.tile_pool(name="gp", bufs=GP_BUFS) as gp,
            tc.tile_pool(name="gwp", bufs=8) as gwp,
            tc.tile_pool(name="obp", bufs=6) as obp,
            tc.tile_pool(name="psp", bufs=4, space="PSUM") as psp,
            tc.tile_pool(name="pst", bufs=2, space="PSUM") as pst,
        ):
            # ---- load inputs / constants to SBUF ----
            rois = cst.tile([R, 5], F32)
            nc.sync.dma_start(rois[:, :], rois_d.ap())
            off = cst.tile([R, 2 * NB], F32)
            nc.sync.dma_start(off[:, :], off_d.ap())
            iopw = cst.tile([R, NB], F32)
            nc.sync.dma_start(iopw[:, :], iopw_d.ap())
            ioph = cst.tile([R, NB], F32)
            nc.sync.dma_start(ioph[:, :], ioph_d.ap())
            iden = cst.tile([R, R], F32)
            nc.sync.dma_start(iden[:, :], iden_d.ap())
            cm = cst.tile([128, NCOL * M2], F32)
            nc.sync.dma_start(cm[:, :], cm_d.ap())

            # gather source: 2 contiguous pixels per desc, row stride 1 pixel
            a0 = data.ap()
            dap = bass.AP(a0.tensor, a0.offset, [[CH, NPX], [1, EL]])

            from contextlib import nullcontext
            loop_cm = tc.For_i(0, reps, 1) if reps > 1 else nullcontext()
            with loop_cm:
                # ---- phase A: per-roi coordinate math, roi on partition ----
                # round(rois[:,1:5]) = floor(x + 0.5)
                rr = wk.tile([R, 4], F32)
                nc.vector.tensor_scalar(rr[:, :], rois[:, 1:5], 0.5, None, OP.add)
                rnd, _ = _floor(nc, wk, rr[:, :], "rnd")

                # start/end in feature coords
                swsh = wk.tile([R, 2], F32)
                nc.vector.tensor_scalar(swsh[:, :], rnd[:, 0:2], SCALE, -0.5, OP.mult, OP.add)
                eweh = wk.tile([R, 2], F32)
                nc.vector.tensor_scalar(
                    eweh[:, :], rnd[:, 2:4], SCALE, SCALE - 0.5, OP.mult, OP.add
                )
                rwh0 = wk.tile([R, 2], F32)
                nc.vector.tensor_tensor(rwh0[:, :], eweh[:, :], swsh[:, :], OP.subtract)
                rwh = wk.tile([R, 2], F32)
                nc.vector.tensor_scalar(rwh[:, :], rwh0[:, :], 0.1, None, OP.max)
                bwh = wk.tile([R, 2], F32)
                nc.vector.tensor_scalar(bwh[:, :], rwh[:, :], 1.0 / P, None, OP.mult)
                swh = wk.tile([R, 2], F32)
                nc.vector.tensor_scalar(swh[:, :], bwh[:, :], 0.5, None, OP.mult)
                rwh01 = wk.tile([R, 2], F32)
                nc.vector.tensor_scalar(rwh01[:, :], rwh[:, :], TRANS_STD, None, OP.mult)
                ybase = wk.tile([R, 1], F32)
                nc.vector.tensor_scalar(ybase[:, :], rois[:, 0:1], float(H * W), None, OP.mult)

                # bin starts, shifted by learned offsets: [R, 49]
                def bin_start(iota, bcol, scol, tview, r01col, name):
                    t0 = wk.tile([R, NB], F32, tag=name + "0")
                    nc.vector.tensor_scalar(t0[:, :], iota, bcol, None, OP.mult)
                    t1 = wk.tile([R, NB], F32, tag=name + "1")
                    nc.vector.scalar_tensor_tensor(
                        t1[:, :], tview, r01col, t0[:, :], OP.mult, OP.add
                    )
                    t2 = wk.tile([R, NB], F32, tag=name + "2")
                    nc.vector.tensor_scalar(t2[:, :], t1[:, :], scol, None, OP.add)
                    return t2

                wstart = bin_start(
                    iopw[:, :], bwh[:, 0:1], swsh[:, 0:1], off[:, 0:NB],
                    rwh01[:, 0:1], "ws",
                )
                hstart = bin_start(
                    ioph[:, :], bwh[:, 1:2], swsh[:, 1:2], off[:, NB : 2 * NB],
                    rwh01[:, 1:2], "hs",
                )

                # sample positions [R, 98] = (bin, s)
                def samples(start, subcol, name):
                    s2 = wk.tile([R, 2 * NB], F32, tag=name)
                    v = s2[:, :].rearrange("p (b s) -> p b s", s=2)
                    su = start[:, :].rearrange("p b -> p b", ).unsqueeze(2)
                    nc.vector.tensor_copy(v[:, :, 0:1], su)
                    nc.vector.tensor_scalar(v[:, :, 1:2], su, subcol, None, OP.add)
                    return s2

                X2 = samples(wstart, swh[:, 0:1], "X2")
                Y2 = samples(hstart, swh[:, 1:2], "Y2")

                # per-axis: validity, clip, floor/frac, corner weight pairs,
                # and (for y only) the clamped corner index pair
                def axis_side(S2, lim, name, want_i4):
                    # valid = (S2 >= -0.5) & (S2 <= lim + 0.5)
                    va = wk.tile([R, 2 * NB], F32, tag=name + "va")
                    nc.vector.tensor_scalar(va[:, :], S2[:, :], -0.5, None, OP.is_ge)
                    vv = wk.tile([R, 2 * NB], F32, tag=name + "vv")
                    nc.vector.scalar_tensor_tensor(
                        vv[:, :], S2[:, :], lim + 0.5, va[:, :], OP.is_le, OP.mult
                    )
                    cl = wk.tile([R, 2 * NB], F32, tag=name + "cl")
                    nc.vector.tensor_scalar(cl[:, :], S2[:, :], 0.0, lim, OP.max, OP.min)
                    flo, fra = _floor(nc, wk, cl[:, :], name + "fl")
                    # count over the 2 samples, per bin -> reciprocal (1 or .5)
                    cnt = wk.tile([R, NB], F32, tag=name + "ct")
                    vvv = vv[:, :].rearrange("p (b s) -> p b s", s=2)
                    nc.vector.tensor_tensor(
                        cnt[:, :].unsqueeze(2),
                        vvv[:, :, 0:1], vvv[:, :, 1:2], OP.add,
                    )
                    eq2 = wk.tile([R, NB], F32, tag=name + "e2")
                    nc.vector.tensor_scalar(eq2[:, :], cnt[:, :], 2.0, None, OP.is_equal)
                    rc = wk.tile([R, NB], F32, tag=name + "rc")
                    nc.vector.tensor_scalar(rc[:, :], eq2[:, :], -0.5, 1.0, OP.mult, OP.add)
                    # weight pair: w0 = v*(1-f)*rc, w1 = v*f*rc  [R, 196] = (b, s, c)
                    rcb = rc[:, :].unsqueeze(2).broadcast_to([R, NB, 2])
                    vr = wk.tile([R, 2 * NB], F32, tag=name + "vr")
                    nc.vector.tensor_tensor(
                        vr[:, :].rearrange("p (b s) -> p b s", s=2), vvv, rcb, OP.mult
                    )
                    w1 = wk.tile([R, 2 * NB], F32, tag=name + "w1")
                    nc.vector.tensor_tensor(w1[:, :], vr[:, :], fra, OP.mult)
                    w0 = wk.tile([R, 2 * NB], F32, tag=name + "w0")
                    nc.vector.tensor_tensor(w0[:, :], vr[:, :], w1[:, :], OP.subtract)
                    W4 = wk.tile([R, 4 * NB], F32, tag=name + "W4")
                    W4v = W4[:, :].rearrange("p (b s c) -> p b s c", s=2, c=2)
                    w0v = w0[:, :].rearrange("p (b s) -> p b s", s=2).unsqueeze(3)
                    w1v = w1[:, :].rearrange("p (b s) -> p b s", s=2).unsqueeze(3)
                    nc.vector.tensor_copy(W4v[:, :, :, 0:1], w0v)
                    nc.vector.tensor_copy(W4v[:, :, :, 1:2], w1v)
                    if not want_i4:
                        return W4, None, flo
                    # index pair: i0 = floor, i1 = min(floor+1, lim)
                    I4 = wk.tile([R, 4 * NB], F32, tag=name + "I4")
                    I4v = I4[:, :].rearrange("p (b s c) -> p b s c", s=2, c=2)
                    flv = flo.rearrange("p (b s) -> p b s", s=2).unsqueeze(3)
                    nc.vector.tensor_copy(I4v[:, :, :, 0:1], flv)
                    nc.vector.tensor_scalar(I4v[:, :, :, 1:2], flv, 1.0, lim, OP.add, OP.min)
                    return W4, I4, flo

                WX4, _, XFL = axis_side(X2, float(W - 1), "x", False)
                WY4, YI4, _ = axis_side(Y2, float(H - 1), "y", True)

                # y-side indices -> flat row base: b*H*W + y*W
                YIr = wk.tile([R, 4 * NB], F32)
                nc.vector.tensor_scalar(
                    YIr[:, :], YI4[:, :], float(W), ybase[:, :], OP.mult, OP.add
                )

                # weights expanded to desc terms, bin-major [R, 392] =
                # (b, h, y, s), one tensor per pixel half (left x1 / right x1+1)
                WX4p = WX4[:, :].rearrange("p (b s c) -> p b c s", s=2, c=2)
                WY4b = (
                    WY4[:, :].rearrange("p (b h y) -> p b h y", h=2, y=2)
                    .unsqueeze(4).broadcast_to([R, NB, 2, 2, 2])
                )
                WtL = wk.tile([R, TD], F32, tag="WtL")
                WtR = wk.tile([R, TD], F32, tag="WtR")
                for Wh, f in ((WtL, 0), (WtR, 1)):
                    Whv = Wh[:, :].rearrange(
                        "p (b h y s) -> p b h y s", h=2, y=2, s=2
                    )
                    nc.vector.tensor_copy(Whv[:, :, :, :, :], WY4b)
                    wxf = WX4p[:, :, f : f + 1, :].unsqueeze(2)
                    for j in range(4):
                        h, y = j >> 1, j & 1
                        dstW = Whv[:, :, h : h + 1, y : y + 1, :]
                        nc.vector.tensor_tensor(dstW, dstW, wxf, OP.mult)

                # descriptor indices lane-major [R, 392] = (h, y, s, b):
                # idx = b*H*W + y_corner*W + floor(x_sample)
                IDX3 = wk.tile([R, TD], F32)
                IDX3v = IDX3[:, :].rearrange(
                    "p (h y s b) -> p h y s b", h=2, y=2, s=2
                )
                YIr2 = (
                    YIr[:, :].rearrange("p (b h y) -> p h y b", h=2, y=2)
                    .unsqueeze(3).broadcast_to([R, 2, 2, 2, NB])
                )
                nc.vector.tensor_copy(IDX3v[:, :, :, :, :], YIr2)
                XFL2 = (
                    XFL.rearrange("p (b s) -> p s b", s=2)
                    .unsqueeze(1).unsqueeze(2)
                )
                for j in range(4):
                    h, y = j >> 1, j & 1
                    dstI = IDX3v[:, h : h + 1, y : y + 1, :, :]
                    nc.vector.tensor_tensor(dstI, dstI, XFL2, OP.add)

                # ---- phase B: int16 gather indices in dma_gather's 16-lane
                # layout: desc i of pair q reads IDXG[i%16, 49q + i//16];
                # i = 16b + 2*t3 + r -> lane 2*t3+r, col = bin.
                IDX16 = wk.tile([R, TD], I16)
                nc.vector.tensor_copy(IDX16[:, :], IDX3[:, :])
                IDXG = wk.tile([128, NPAIR * NB], I16)
                for t3 in range(T3):
                    for r in range(2):
                        nc.sync.dma_start(
                            IDXG[2 * t3 + r : 2 * t3 + r + 1, :],
                            IDX16[32 * r : 32 * r + 32, t3 * NB : (t3 + 1) * NB],
                        )
                # Q7 tx/rx cpus each read their own 16-partition window of the
                # index tensor -> replicate lane group 0 across all 8 groups.
                for grp in range(1, 8):
                    nc.sync.dma_start(
                        IDXG[16 * grp : 16 * (grp + 1), :], IDXG[0:16, :]
                    )

                # Weights at gather partitions: desc d = 128c + p with
                # p = 2*(8*(b%8) + t3) + r, so transposing WtL/WtR in 64-wide
                # windows (j = 8*(b%8)+t3 contiguous per column) and writing
                # with one stride-2-partition DMA per (half, roi-half) puts
                # weight j at partition 2j + r.  Staged per (half, r) so it
                # is 4 DMAs total.
                WL = wk.tile([128, NCOL * NPAIR], F32, tag="WL")
                WR = wk.tile([128, NCOL * NPAIR], F32, tag="WR")
                nc.vector.memset(WL[:, :], 0.0)
                nc.vector.memset(WR[:, :], 0.0)
                for Wh, Wsrc, tag in ((WL, WtL, "L"), (WR, WtR, "R")):
                    st0 = wk.tile([64, NCOL * NPAIR], F32, tag="st0" + tag)
                    st1 = wk.tile([64, NCOL * NPAIR], F32, tag="st1" + tag)
                    # col 6 rows 8.. (-> Wh partitions 16..) never gathered:
                    # keep finite zeros
                    nc.vector.memset(st0[:, 6 * NPAIR :], 0.0)
                    nc.vector.memset(st1[:, 6 * NPAIR :], 0.0)
                    for c in range(NCOL):
                        n = 64 if c < 6 else 8
                        ps = pst.tile([64, R], F32, tag="pstr")
                        nc.tensor.transpose(
                            ps[0:n, :], Wsrc[:, 64 * c : 64 * c + n], iden[:, :]
                        )
                        nc.vector.tensor_copy(
                            st0[0:n, c * NPAIR : (c + 1) * NPAIR], ps[0:n, 0:32]
                        )
                        nc.vector.tensor_copy(
                            st1[0:n, c * NPAIR : (c + 1) * NPAIR], ps[0:n, 32:64]
                        )
                    nc.sync.dma_start(Wh[0::2, :], st0[:, :])
                    nc.sync.dma_start(Wh[1::2, :], st1[:, :])

                # ---- phase C: gather + weighted reduce, one pair at a time --
                out_r = out_d.ap()
                for q in range(NPAIR):
                    gt = gp.tile([128, NCOL * EL], BF16)
                    if q < GP_BUFS or bench_mode == 2:
                        # col 6 partitions 16.. are never gathered; clear on
                        # first use so masked-0 products stay finite.
                        nc.vector.memset(gt[:, 6 * EL : 7 * EL], 0.0)
                    if bench_mode != 2:
                        dest = gt[:, :].rearrange("p (j f) -> p j f", f=EL)
                        nc.gpsimd.dma_gather(
                            dest,
                            dap,
                            IDXG[:, q * NB : (q + 1) * NB],
                            ND,
                            ND,
                            EL,
                            elem_step=CH,
                            queue_num=q % 4,
                        )
                    if bench_mode == 1:
                        continue
                    # weighted masks (bf16) for left / right pixels
                    wmL = gwp.tile([128, NCOL * M2], BF16, tag="wmL")
                    wmR = gwp.tile([128, NCOL * M2], BF16, tag="wmR")
                    for wm, wsrc in ((wmL, WL), (wmR, WR)):
                        wtb = (
                            wsrc[:, :].rearrange("p (c q) -> p c q", q=NPAIR)
                            [:, :, q : q + 1].broadcast_to([128, NCOL, M2])
                        )
                        nc.any.tensor_tensor(
                            wm[:, :].rearrange("p (c j) -> p c j", j=M2),
                            cm[:, :].rearrange("p (c j) -> p c j", j=M2),
                            wtb,
                            OP.mult,
                        )
                    ps = psp.tile([M2, CH], F32)
                    for c in range(NCOL):
                        nc.tensor.matmul(
                            ps[:, :],
                            wmL[:, c * M2 : (c + 1) * M2],
                            gt[:, c * EL : c * EL + CH],
                            start=(c == 0),
                            stop=False,
                        )
                        nc.tensor.matmul(
                            ps[:, :],
                            wmR[:, c * M2 : (c + 1) * M2],
                            gt[:, c * EL + CH : (c + 1) * EL],
                            start=False,
                            stop=(c == NCOL - 1),
                        )
                    ob = obp.tile([M2, CH], F32)
                    nc.scalar.copy(ob[:, :], ps[:, :])
                    nc.sync.dma_start(out_r[q : q + 1, :], ob[0:NB, :])
                    nc.sync.dma_start(out_r[q + 32 : q + 33, :], ob[NB:M2, :])

    nc.finalize()
    return nc


def host_constants():
    iopw = np.tile((np.arange(NB) % P).astype(np.float32), (R, 1))
    ioph = np.tile((np.arange(NB) // P).astype(np.float32), (R, 1))
    iden = np.eye(R, dtype=np.float32)
    # mask from the descriptor stream: desc i = 16b + 2*t3 + r
    cm = np.zeros((128, NCOL * M2), dtype=np.float32)
    for b in range(NB):
        for t3 in range(T3):
            for r in range(2):
                i = 16 * b + 2 * t3 + r
                cm[i % 128, (i // 128) * M2 + NB * r + b] = 1.0
    return {"iota_pw": iopw, "iota_ph": ioph, "identity": iden, "cmask": cm}


_cache = {}


def _program():
    if "nc" not in _cache:
        _cache["nc"] = build_program()
    return _cache["nc"]


def run(data, rois, offset, **spmd_kwargs):
    data = np.asarray(data, dtype=np.float32)
    rois = np.asarray(rois, dtype=np.float32)
    offset = np.asarray(offset, dtype=np.float32)
    n_rois = rois.shape[0]
    data_t = (
        np.ascontiguousarray(data.transpose(0, 2, 3, 1))
        .reshape(NPX, CH)
        .astype(ml_dtypes.bfloat16)
    )
    data_t = np.concatenate(
        [data_t, np.zeros((PAD, CH), dtype=ml_dtypes.bfloat16)], axis=0
    )
    consts = host_constants()
    in_maps = []
    for c in range(N_CORES):
        sl = slice(c * R, (c + 1) * R)
        m = {
            "data_t": data_t,
            "rois": rois[sl],
            "offs": offset[sl].reshape(R, 2 * NB),
        }
        m.update(consts)
        in_maps.append(m)
    res = run_bass_kernel_spmd(
        _program(), in_maps, core_ids=list(range(N_CORES)), **spmd_kwargs
    )
    outs = np.concatenate([res.results[c]["out"] for c in range(N_CORES)], axis=0)
    out = outs.reshape(n_rois, NB, CH).transpose(0, 2, 1).reshape(n_rois, CH, P, P)
    return np.ascontiguousarray(out), res


def kernel(data, rois, offset):
    out, _ = run(data, rois, offset)
    return out


# revision 26
# speedup vs baseline: 1.5942x; 1.0986x over previous
"""Deformable PS-ROI pooling on Trainium2 (Bass/Tile), SPMD over 8 cores.

Strategy: data-parallel over ROIs (64 rois/core), feature map replicated in
DRAM in channel-last bf16 layout.  The two x-corners of a bilinear sample
are always adjacent pixels (x1, x1+1), so each gather descriptor fetches 2
contiguous pixels (1 KiB); HW gather cost is descriptor-bound, so this
halves gather time vs per-pixel descriptors.  ROIs are processed in pairs
(q, q+32): one 784-descriptor dma_gather per pair (fits the ~1024-desc Q7
idx scratch cap).  Descriptor i = 16*bin + 2*t3 + r (t3 = sample_h x
corner_y x sample_w, r = roi half) lands at partition 16*(bin%8)+2*t3+r,
so a single mask constant works for every column and the 16-lane index
tensor is a plain per-lane DMA.  A [128, 98] bf16 mask matmul on the PE
reduces each landing column into the pair's [98, 256] psum (rows 0..48
roi q, 49..97 roi q+32), with all bilinear / validity / 1-over-count
factors pre-folded into per-partition scalar weights (separate left- and
right-pixel variants, placed by stride-2-partition DMAs).
"""

import numpy as np
import ml_dtypes

import concourse.bass as bass
import concourse.bacc as bacc
import concourse.mybir as mybir
from concourse import tile
from concourse.bass_utils import run_bass_kernel_spmd

F32 = mybir.dt.float32
BF16 = mybir.dt.bfloat16
I32 = mybir.dt.int32
I16 = mybir.dt.int16
OP = mybir.AluOpType

N_CORES = 8
R = 64                  # rois per core
P = 7                   # pooled output size
NB = P * P              # 49 bins
CH = 256                # channels
H = W = 128             # feature map spatial
B = 2                   # batch
NPX = B * H * W         # 32768 flat pixels
PAD = 4                 # extra zero pixels (right-px overrun at x1=W-1)
T3 = 8                  # terms per (bin, roi): sample_h x corner_y x sample_w
TD = NB * T3            # 392 descriptor-terms per roi
ND = 2 * TD             # 784 descriptors per pair
NPAIR = R // 2
NCOL = 7                # gather dest cols per pair (784 = 6*128 + 16)
EL = 2 * CH             # elements per descriptor (2 pixels)
M2 = 2 * NB             # 98 psum rows per pair
SCALE = 0.0625
TRANS_STD = 0.1
GP_BUFS = 8


def _floor(nc, pool, x, name):
    """floor(x) robust to convert rounding mode: returns (floor_f32, frac)."""
    xi = pool.tile([R, x.shape[1]], I32, tag=name + "_i")
    nc.vector.tensor_copy(xi[:, :], x)
    xf = pool.tile([R, x.shape[1]], F32, tag=name + "_f")
    nc.vector.tensor_copy(xf[:, :], xi[:, :])
    d = pool.tile([R, x.shape[1]], F32, tag=name + "_d")
    nc.vector.tensor_tensor(d[:, :], x, xf[:, :], OP.subtract)
    neg = pool.tile([R, x.shape[1]], F32, tag=name + "_n")
    nc.vector.tensor_scalar(neg[:, :], d[:, :], 0.0, None, OP.is_lt)
    fl = pool.tile([R, x.shape[1]], F32, tag=name + "_fl")
    nc.vector.tensor_tensor(fl[:, :], xf[:, :], neg[:, :], OP.subtract)
    fr = pool.tile([R, x.shape[1]], F32, tag=name + "_fr")
    nc.vector.tensor_tensor(fr[:, :], d[:, :], neg[:, :], OP.add)
    return fl[:, :], fr[:, :]


def build_program(reps: int = 1, bench_mode: int = 0):
    """bench_mode: 0=full kernel, 1=gathers only (no reduce), 2=no gathers."""
    nc = bacc.Bacc("TRN2", target_bir_lowering=False, debug=False, num_swdge_queues=4)
    nc.dynamic_dma_scratch_size = 2 ** 16

    data = nc.dram_tensor("data_t", [NPX + PAD, CH], BF16, kind="ExternalInput")
    rois_d = nc.dram_tensor("rois", [R, 5], F32, kind="ExternalInput")
    off_d = nc.dram_tensor("offs", [R, 2 * NB], F32, kind="ExternalInput")
    iopw_d = nc.dram_tensor("iota_pw", [R, NB], F32, kind="ExternalInput")
    ioph_d = nc.dram_tensor("iota_ph", [R, NB], F32, kind="ExternalInput")
    iden_d = nc.dram_tensor("identity", [R, R], F32, kind="ExternalInput")
    cm_d = nc.dram_tensor("cmask", [128, NCOL * M2], F32, kind="ExternalInput")
    out_d = nc.dram_tensor("out", [R, NB * CH], F32, kind="ExternalOutput")

    with tile.TileContext(nc) as tc:
        with (
            tc.tile_pool(name="const", bufs=1) as cst,
            tc.tile_pool(name="work", bufs=1) as wk,
            tc.tile_pool(name="gp", bufs=GP_BUFS) as gp,
            tc.tile_pool(name="gwp", bufs=8) as gwp,
            tc.tile_pool(name="obp", bufs=3) as obp,
            tc.tile_pool(name="psp", bufs=4, space="PSUM") as psp,
            tc.tile_pool(name="pst", bufs=2, space="PSUM") as pst,
        ):
            # ---- load inputs / constants to SBUF ----
            rois = cst.tile([R, 5], F32)
            nc.sync.dma_start(rois[:, :], rois_d.ap())
            off = cst.tile([R, 2 * NB], F32)
            nc.sync.dma_start(off[:, :], off_d.ap())
            iopw = cst.tile([R, NB], F32)
            nc.sync.dma_start(iopw[:, :], iopw_d.ap())
            ioph = cst.tile([R, NB], F32)
            nc.sync.dma_start(ioph[:, :], ioph_d.ap())
            iden = cst.tile([R, R], F32)
            nc.sync.dma_start(iden[:, :], iden_d.ap())
            cm = cst.tile([128, NCOL * M2], F32)
            nc.sync.dma_start(cm[:, :], cm_d.ap())

            # gather source: 2 contiguous pixels per desc, row stride 1 pixel
            a0 = data.ap()
            dap = bass.AP(a0.tensor, a0.offset, [[CH, NPX], [1, EL]])

            from contextlib import nullcontext
            loop_cm = tc.For_i(0, reps, 1) if reps > 1 else nullcontext()
            with loop_cm:
                # ---- phase A: per-roi coordinate math, roi on partition ----
                # round(rois[:,1:5]) = floor(x + 0.5)
                rr = wk.tile([R, 4], F32)
                nc.vector.tensor_scalar(rr[:, :], rois[:, 1:5], 0.5, None, OP.add)
                rnd, _ = _floor(nc, wk, rr[:, :], "rnd")

                # start/end in feature coords
                swsh = wk.tile([R, 2], F32)
                nc.vector.tensor_scalar(swsh[:, :], rnd[:, 0:2], SCALE, -0.5, OP.mult, OP.add)
                eweh = wk.tile([R, 2], F32)
                nc.vector.tensor_scalar(
                    eweh[:, :], rnd[:, 2:4], SCALE, SCALE - 0.5, OP.mult, OP.add
                )
                rwh0 = wk.tile([R, 2], F32)
                nc.vector.tensor_tensor(rwh0[:, :], eweh[:, :], swsh[:, :], OP.subtract)
                rwh = wk.tile([R, 2], F32)
                nc.vector.tensor_scalar(rwh[:, :], rwh0[:, :], 0.1, None, OP.max)
                bwh = wk.tile([R, 2], F32)
                nc.vector.tensor_scalar(bwh[:, :], rwh[:, :], 1.0 / P, None, OP.mult)
                swh = wk.tile([R, 2], F32)
                nc.vector.tensor_scalar(swh[:, :], bwh[:, :], 0.5, None, OP.mult)
                rwh01 = wk.tile([R, 2], F32)
                nc.vector.tensor_scalar(rwh01[:, :], rwh[:, :], TRANS_STD, None, OP.mult)
                ybase = wk.tile([R, 1], F32)
                nc.vector.tensor_scalar(ybase[:, :], rois[:, 0:1], float(H * W), None, OP.mult)

                # bin starts, shifted by learned offsets: [R, 49]
                def bin_start(iota, bcol, scol, tview, r01col, name):
                    t0 = wk.tile([R, NB], F32, tag=name + "0")
                    nc.vector.tensor_scalar(t0[:, :], iota, bcol, None, OP.mult)
                    t1 = wk.tile([R, NB], F32, tag=name + "1")
                    nc.vector.scalar_tensor_tensor(
                        t1[:, :], tview, r01col, t0[:, :], OP.mult, OP.add
                    )
                    t2 = wk.tile([R, NB], F32, tag=name + "2")
                    nc.vector.tensor_scalar(t2[:, :], t1[:, :], scol, None, OP.add)
                    return t2

                wstart = bin_start(
                    iopw[:, :], bwh[:, 0:1], swsh[:, 0:1], off[:, 0:NB],
                    rwh01[:, 0:1], "ws",
                )
                hstart = bin_start(
                    ioph[:, :], bwh[:, 1:2], swsh[:, 1:2], off[:, NB : 2 * NB],
                    rwh01[:, 1:2], "hs",
                )

                # sample positions [R, 98] = (bin, s)
                def samples(start, subcol, name):
                    s2 = wk.tile([R, 2 * NB], F32, tag=name)
                    v = s2[:, :].rearrange("p (b s) -> p b s", s=2)
                    su = start[:, :].rearrange("p b -> p b", ).unsqueeze(2)
                    nc.vector.tensor_copy(v[:, :, 0:1], su)
                    nc.vector.tensor_scalar(v[:, :, 1:2], su, subcol, None, OP.add)
                    return s2

                X2 = samples(wstart, swh[:, 0:1], "X2")
                Y2 = samples(hstart, swh[:, 1:2], "Y2")

                # per-axis: validity, clip, floor/frac, corner weight pairs,
                # and (for y only) the clamped corner index pair
                def axis_side(S2, lim, name, want_i4):
                    # valid = (S2 >= -0.5) & (S2 <= lim + 0.5)
                    va = wk.tile([R, 2 * NB], F32, tag=name + "va")
                    nc.vector.tensor_scalar(va[:, :], S2[:, :], -0.5, None, OP.is_ge)
                    vv = wk.tile([R, 2 * NB], F32, tag=name + "vv")
                    nc.vector.scalar_tensor_tensor(
                        vv[:, :], S2[:, :], lim + 0.5, va[:, :], OP.is_le, OP.mult
                    )
                    cl = wk.tile([R, 2 * NB], F32, tag=name + "cl")
                    nc.vector.tensor_scalar(cl[:, :], S2[:, :], 0.0, lim, OP.max, OP.min)
                    flo, fra = _floor(nc, wk, cl[:, :], name + "fl")
                    # count over the 2 samples, per bin -> reciprocal (1 or .5)
                    cnt = wk.tile([R, NB], F32, tag=name + "ct")
                    vvv = vv[:, :].rearrange("p (b s) -> p b s", s=2)
                    nc.vector.tensor_tensor(
                        cnt[:, :].unsqueeze(2),
                        vvv[:, :, 0:1], vvv[:, :, 1:2], OP.add,
                    )
                    eq2 = wk.tile([R, NB], F32, tag=name + "e2")
                    nc.vector.tensor_scalar(eq2[:, :], cnt[:, :], 2.0, None, OP.is_equal)
                    rc = wk.tile([R, NB], F32, tag=name + "rc")
                    nc.vector.tensor_scalar(rc[:, :], eq2[:, :], -0.5, 1.0, OP.mult, OP.add)
                    # weight pair: w0 = v*(1-f)*rc, w1 = v*f*rc  [R, 196] = (b, s, c)
                    rcb = rc[:, :].unsqueeze(2).broadcast_to([R, NB, 2])
                    vr = wk.tile([R, 2 * NB], F32, tag=name + "vr")
                    nc.vector.tensor_tensor(
                        vr[:, :].rearrange("p (b s) -> p b s", s=2), vvv, rcb, OP.mult
                    )
                    w1 = wk.tile([R, 2 * NB], F32, tag=name + "w1")
                    nc.vector.tensor_tensor(w1[:, :], vr[:, :], fra, OP.mult)
                    w0 = wk.tile([R, 2 * NB], F32, tag=name + "w0")
                    nc.vector.tensor_tensor(w0[:, :], vr[:, :], w1[:, :], OP.subtract)
                    W4 = wk.tile([R, 4 * NB], F32, tag=name + "W4")
                    W4v = W4[:, :].rearrange("p (b s c) -> p b s c", s=2, c=2)
                    w0v = w0[:, :].rearrange("p (b s) -> p b s", s=2).unsqueeze(3)
                    w1v = w1[:, :].rearrange("p (b s) -> p b s", s=2).unsqueeze(3)
                    nc.vector.tensor_copy(W4v[:, :, :, 0:1], w0v)
                    nc.vector.tensor_copy(W4v[:, :, :, 1:2], w1v)
                    if not want_i4:
                        return W4, None, flo
                    # index pair: i0 = floor, i1 = min(floor+1, lim)
                    I4 = wk.tile([R, 4 * NB], F32, tag=name + "I4")
                    I4v = I4[:, :].rearrange("p (b s c) -> p b s c", s=2, c=2)
                    flv = flo.rearrange("p (b s) -> p b s", s=2).unsqueeze(3)
                    nc.vector.tensor_copy(I4v[:, :, :, 0:1], flv)
                    nc.vector.tensor_scalar(I4v[:, :, :, 1:2], flv, 1.0, lim, OP.add, OP.min)
                    return W4, I4, flo

                WX4, _, XFL = axis_side(X2, float(W - 1), "x", False)
                WY4, YI4, _ = axis_side(Y2, float(H - 1), "y", True)

                # y-side indices -> flat row base: b*H*W + y*W
                YIr = wk.tile([R, 4 * NB], F32)
                nc.vector.tensor_scalar(
                    YIr[:, :], YI4[:, :], float(W), ybase[:, :], OP.mult, OP.add
                )

                # weights expanded to desc terms, bin-major [R, 392] =
                # (b, h, y, s), one tensor per pixel half (left x1 / right x1+1)
                WX4p = WX4[:, :].rearrange("p (b s c) -> p b c s", s=2, c=2)
                WY4b = (
                    WY4[:, :].rearrange("p (b h y) -> p b h y", h=2, y=2)
                    .unsqueeze(4).broadcast_to([R, NB, 2, 2, 2])
                )
                WtL = wk.tile([R, TD], F32, tag="WtL")
                WtR = wk.tile([R, TD], F32, tag="WtR")
                for Wh, f in ((WtL, 0), (WtR, 1)):
                    Whv = Wh[:, :].rearrange(
                        "p (b h y s) -> p b h y s", h=2, y=2, s=2
                    )
                    nc.vector.tensor_copy(Whv[:, :, :, :, :], WY4b)
                    wxf = WX4p[:, :, f : f + 1, :].unsqueeze(2)
                    for j in range(4):
                        h, y = j >> 1, j & 1
                        dstW = Whv[:, :, h : h + 1, y : y + 1, :]
                        nc.vector.tensor_tensor(dstW, dstW, wxf, OP.mult)

                # descriptor indices lane-major [R, 392] = (h, y, s, b):
                # idx = b*H*W + y_corner*W + floor(x_sample)
                IDX3 = wk.tile([R, TD], F32)
                IDX3v = IDX3[:, :].rearrange(
                    "p (h y s b) -> p h y s b", h=2, y=2, s=2
                )
                YIr2 = (
                    YIr[:, :].rearrange("p (b h y) -> p h y b", h=2, y=2)
                    .unsqueeze(3).broadcast_to([R, 2, 2, 2, NB])
                )
                nc.vector.tensor_copy(IDX3v[:, :, :, :, :], YIr2)
                XFL2 = (
                    XFL.rearrange("p (b s) -> p s b", s=2)
                    .unsqueeze(1).unsqueeze(2)
                )
                for j in range(4):
                    h, y = j >> 1, j & 1
                    dstI = IDX3v[:, h : h + 1, y : y + 1, :, :]
                    nc.vector.tensor_tensor(dstI, dstI, XFL2, OP.add)

                # ---- phase B: int16 gather indices in dma_gather's 16-lane
                # layout: desc i of pair q reads IDXG[i%16, 49q + i//16];
                # i = 16b + 2*t3 + r -> lane 2*t3+r, col = bin.
                IDX16 = wk.tile([R, TD], I16)
                nc.vector.tensor_copy(IDX16[:, :], IDX3[:, :])
                IDXG = wk.tile([128, NPAIR * NB], I16)
                for t3 in range(T3):
                    for r in range(2):
                        nc.sync.dma_start(
                            IDXG[2 * t3 + r : 2 * t3 + r + 1, :],
                            IDX16[32 * r : 32 * r + 32, t3 * NB : (t3 + 1) * NB],
                        )
                # Q7 tx/rx cpus each read their own 16-partition window of the
                # index tensor -> replicate lane group 0 across all 8 groups.
                for grp in range(1, 8):
                    nc.sync.dma_start(
                        IDXG[16 * grp : 16 * (grp + 1), :], IDXG[0:16, :]
                    )

                # Weights at gather partitions: desc d = 128c + p with
                # p = 2*(8*(b%8) + t3) + r, so transposing WtL/WtR in 64-wide
                # windows (j = 8*(b%8)+t3 contiguous per column) and writing
                # with one stride-2-partition DMA per (half, roi-half) puts
                # weight j at partition 2j + r.  Staged per (half, r) so it
                # is 4 DMAs total.
                WL = wk.tile([128, NCOL * NPAIR], F32, tag="WL")
                WR = wk.tile([128, NCOL * NPAIR], F32, tag="WR")
                nc.vector.memset(WL[:, :], 0.0)
                nc.vector.memset(WR[:, :], 0.0)
                for Wh, Wsrc, tag in ((WL, WtL, "L"), (WR, WtR, "R")):
                    st0 = wk.tile([64, NCOL * NPAIR], F32, tag="st0" + tag)
                    st1 = wk.tile([64, NCOL * NPAIR], F32, tag="st1" + tag)
                    # col 6 rows 8.. (-> Wh partitions 16..) never gathered:
                    # keep finite zeros
                    nc.vector.memset(st0[:, 6 * NPAIR :], 0.0)
                    nc.vector.memset(st1[:, 6 * NPAIR :], 0.0)
                    for c in range(NCOL):
                        n = 64 if c < 6 else 8
                        ps = pst.tile([64, R], F32, tag="pstr")
                        nc.tensor.transpose(
                            ps[0:n, :], Wsrc[:, 64 * c : 64 * c + n], iden[:, :]
                        )
                        nc.vector.tensor_copy(
                            st0[0:n, c * NPAIR : (c + 1) * NPAIR], ps[0:n, 0:32]
                        )
                        nc.vector.tensor_copy(
                            st1[0:n, c * NPAIR : (c + 1) * NPAIR], ps[0:n, 32:64]
                        )
                    nc.sync.dma_start(Wh[0::2, :], st0[:, :])
                    nc.sync.dma_start(Wh[1::2, :], st1[:, :])

                # ---- phase C: gather + weighted reduce, one pair at a time --
                out_v = out_d.ap().rearrange("r (b c) -> b r c", c=CH)
                for q in range(NPAIR):
                    gt = gp.tile([128, NCOL * EL], BF16)
                    if q < GP_BUFS or bench_mode == 2:
                        # col 6 partitions 16.. are never gathered; clear on
                        # first use so masked-0 products stay finite.
                        nc.vector.memset(gt[:, 6 * EL : 7 * EL], 0.0)
                    if bench_mode != 2:
                        dest = gt[:, :].rearrange("p (j f) -> p j f", f=EL)
                        nc.gpsimd.dma_gather(
                            dest,
                            dap,
                            IDXG[:, q * NB : (q + 1) * NB],
                            ND,
                            ND,
                            EL,
                            elem_step=CH,
                            queue_num=q % 4,
                        )
                    if bench_mode == 1:
                        continue
                    # weighted masks (bf16) for left / right pixels
                    wmL = gwp.tile([128, NCOL * M2], BF16, tag="wmL")
                    wmR = gwp.tile([128, NCOL * M2], BF16, tag="wmR")
                    for wm, wsrc in ((wmL, WL), (wmR, WR)):
                        wtb = (
                            wsrc[:, :].rearrange("p (c q) -> p c q", q=NPAIR)
                            [:, :, q : q + 1].broadcast_to([128, NCOL, M2])
                        )
                        nc.any.tensor_tensor(
                            wm[:, :].rearrange("p (c j) -> p c j", j=M2),
                            cm[:, :].rearrange("p (c j) -> p c j", j=M2),
                            wtb,
                            OP.mult,
                        )
                    ps = psp.tile([M2, CH], F32)
                    for c in range(NCOL):
                        nc.tensor.matmul(
                            ps[:, :],
                            wmL[:, c * M2 : (c + 1) * M2],
                            gt[:, c * EL : c * EL + CH],
                            start=(c == 0),
                            stop=False,
                        )
                        nc.tensor.matmul(
                            ps[:, :],
                            wmR[:, c * M2 : (c + 1) * M2],
                            gt[:, c * EL + CH : (c + 1) * EL],
                            start=False,
                            stop=(c == NCOL - 1),
                        )
                    # outputs batched 4 pairs per DMA (rows q0..q0+3 and
                    # q0+32..q0+35)
                    if q % 4 == 0:
                        ob = obp.tile([M2, 4 * CH], F32, tag="ob")
                    k = q % 4
                    nc.scalar.copy(ob[:, k * CH : (k + 1) * CH], ps[:, :])
                    if q % 4 == 3:
                        q0 = q - 3
                        nc.sync.dma_start(
                            out_v[:, q0 : q0 + 4, :],
                            ob[0:NB, :].rearrange("p (r c) -> p r c", c=CH),
                        )
                        nc.sync.dma_start(
                            out_v[:, q0 + 32 : q0 + 36, :],
                            ob[NB:M2, :].rearrange("p (r c) -> p r c", c=CH),
                        )

    nc.finalize()
    return nc


def host_constants():
    iopw = np.tile((np.arange(NB) % P).astype(np.float32), (R, 1))
    ioph = np.tile((np.arange(NB) // P).astype(np.float32), (R, 1))
    iden = np.eye(R, dtype=np.float32)
    # mask from the descriptor stream: desc i = 16b + 2*t3 + r
    cm = np.zeros((128, NCOL * M2), dtype=np.float32)
    for b in range(NB):
        for t3 in range(T3):
            for r in range(2):
                i = 16 * b + 2 * t3 + r
                cm[i % 128, (i // 128) * M2 + NB * r + b] = 1.0
    return {"iota_pw": iopw, "iota_ph": ioph, "identity": iden, "cmask": cm}


_cache = {}


def _program():
    if "nc" not in _cache:
        _cache["nc"] = build_program()
    return _cache["nc"]


def run(data, rois, offset, **spmd_kwargs):
    data = np.asarray(data, dtype=np.float32)
    rois = np.asarray(rois, dtype=np.float32)
    offset = np.asarray(offset, dtype=np.float32)
    n_rois = rois.shape[0]
    data_t = (
        np.ascontiguousarray(data.transpose(0, 2, 3, 1))
        .reshape(NPX, CH)
        .astype(ml_dtypes.bfloat16)
    )
    data_t = np.concatenate(
        [data_t, np.zeros((PAD, CH), dtype=ml_dtypes.bfloat16)], axis=0
    )
    consts = host_constants()
    in_maps = []
    for c in range(N_CORES):
        sl = slice(c * R, (c + 1) * R)
        m = {
            "data_t": data_t,
            "rois": rois[sl],
            "offs": offset[sl].reshape(R, 2 * NB),
        }
        m.update(consts)
        in_maps.append(m)
    res = run_bass_kernel_spmd(
        _program(), in_maps, core_ids=list(range(N_CORES)), **spmd_kwargs
    )
    outs = np.concatenate([res.results[c]["out"] for c in range(N_CORES)], axis=0)
    out = outs.reshape(n_rois, NB, CH).transpose(0, 2, 1).reshape(n_rois, CH, P, P)
    return np.ascontiguousarray(out), res


def kernel(data, rois, offset):
    out, _ = run(data, rois, offset)
    return out
